# revision 1
# baseline (speedup 1.0000x reference)
"""AttnBlock (channel attention over 64x64 maps) for Trainium2 — Gram edition.

Data-parallel over batch: 16 batches, 2 per core on 8 NeuronCores.
Per batch [C=512, N=4096], hn = A*x + B (GroupNorm folded to per-channel
affine):

  scores = q^T k with q = Wq hn + bq factorizes through the Gram matrix
    G = (A*x) @ (A*x)^T  (C x C):
      scores = (16Wq) G (16Wk)^T / 65536  + rank-1 corrections
    where the corrections come from the per-channel rowsums (free from the
    bn_stats means) and the GroupNorm shift B. This replaces the q-proj,
    k-proj and scores passes (3 full C*C*N GEMMs + their PSUM evictions)
    with: a diagonal matmul building pixel-major hnT (16k cyc), the Gram
    GEMM (upper triangle + mirrored blocks, ~41k cyc), and two C*C*C GEMMs
    (8k cyc each). All bf16 with fp32 PSUM accumulation.
  v / attn@v / out-proj keep the proven bf16 layout: Wv's columns scaled
    by A so v projects straight from the resident bf16 x; softmax is
    max-subtracted; residual re-reads fp32 x slices.
Batches are software-pipelined; v-projection is split around the scores
GEMM to cover the T1-transpose DMA seam.
"""

import sys

if "/opt/trn_rl_repo" not in sys.path:
    sys.path.insert(0, "/opt/trn_rl_repo")

import numpy as np

C = 512          # channels
N = 4096         # pixels (64*64)
BB = 2           # batches per core
P = 128          # partitions
CB = C // P      # 4 channel blocks
NT = N // P      # 32 pixel tiles of 128
NTH = 8          # pixel tiles per hnT chunk
NSL = 512        # pixel slice width (v / ef phases)
NS = N // NSL    # 8 pixel slices
GROUPS = 32
EPS = 1e-6
SCALE = float(C) ** -0.5
SC2 = SCALE / 65536.0
LN128 = float(np.log(128.0))

_NC_CACHE = {}
LAST_RESULT = None


def _build_nc():
    import concourse.bacc as bacc
    import concourse.tile as tile
    from concourse import mybir
    from concourse.bass import ts

    F32 = mybir.dt.float32
    BF16 = mybir.dt.bfloat16
    AX = mybir.AxisListType
    AF = mybir.ActivationFunctionType
    OP = mybir.AluOpType

    nc = bacc.Bacc(None, target_bir_lowering=False, num_swdge_queues=4)

    xsb_d = nc.dram_tensor("xsb", [BB, C, N], BF16, kind="ExternalInput")
    wqt16_d = nc.dram_tensor("wqt16", [C, C], BF16, kind="ExternalInput")
    wkt16_d = nc.dram_tensor("wkt16", [C, C], BF16, kind="ExternalInput")
    wvt_d = nc.dram_tensor("wvtb", [C, C], BF16, kind="ExternalInput")
    wot_d = nc.dram_tensor("wotb", [C, C], BF16, kind="ExternalInput")
    bq256_d = nc.dram_tensor("bq256", [C], F32, kind="ExternalInput")
    bk256_d = nc.dram_tensor("bk256", [C], F32, kind="ExternalInput")
    bv_d = nc.dram_tensor("bv", [C], F32, kind="ExternalInput")
    bo_d = nc.dram_tensor("bo", [C], F32, kind="ExternalInput")
    gamma_d = nc.dram_tensor("gamma", [C], F32, kind="ExternalInput")
    beta_d = nc.dram_tensor("beta", [C], F32, kind="ExternalInput")
    gfwd_d = nc.dram_tensor("gfwd", [P, CB, GROUPS], F32, kind="ExternalInput")
    gbwd_d = nc.dram_tensor("gbwd", [GROUPS, CB, P], F32, kind="ExternalInput")
    identbf_d = nc.dram_tensor("identbf", [P, P], BF16, kind="ExternalInput")
    y_d = nc.dram_tensor("y", [BB, C, N], F32, kind="ExternalOutput")

    with tile.TileContext(nc) as tc:
        with (
            tc.tile_pool(name="singles", bufs=1) as sg,
            tc.tile_pool(name="sbp", bufs=1) as sbp,
            tc.tile_pool(name="psp", bufs=1, space="PSUM") as psp,
            tc.tile_pool(name="drp", bufs=1, space="DRAM") as drp,
        ):
            xbview = [xsb_d[b].rearrange("(cb p) n -> p cb n", p=P) for b in range(BB)]
            yview = [y_d[b].rearrange("(ob p) n -> p ob n", p=P) for b in range(BB)]
            st = [dict() for _ in range(BB)]  # per-batch tile state

            def emit_load(b):
                s = st[b]
                xbf = sbp.tile([P, CB, N], BF16, tag="xbf", bufs=2, name=f"xbf{b}")
                s["xbf"] = xbf
                for cb in range(CB):
                    nc.sync.dma_start(xbf[:, cb, :], xbview[b][:, cb, :])

            def emit_stats(b, split=False):
                """Per-channel [mean, E[x^2]] -> t."""
                s = st[b]
                xbf = s["xbf"]
                t = sbp.tile([P, CB, 2], F32, tag="t", bufs=2, name=f"t{b}")
                act_cbs = (0, 1) if split else ()
                bn_cbs = [cb for cb in range(CB) if cb not in act_cbs]
                stats = sbp.tile(
                    [P, CB, 8, 6], F32, tag="stats", bufs=2, name=f"st{b}"
                )
                mv = sbp.tile([P, CB, 2], F32, tag="mv", bufs=2, name=f"mv{b}")
                for cb in act_cbs:
                    # scratch shares the (not-yet-written) vfull buffer
                    sq = sbp.tile([P, N], F32, tag="vfull", bufs=1,
                                  name=f"sq{b}{cb}")
                    s1 = sbp.tile([P, 1], F32, tag="s1", bufs=2, name=f"s1{b}{cb}")
                    s2 = sbp.tile([P, 1], F32, tag="s2", bufs=2, name=f"s2{b}{cb}")
                    nc.scalar.activation(
                        sq, xbf[:, cb, :], AF.Copy, accum_out=s1
                    )
                    nc.scalar.activation(
                        sq, xbf[:, cb, :], AF.Square, accum_out=s2
                    )
                    nc.vector.tensor_scalar_mul(t[:, cb, 0:1], s1, 1.0 / N)
                    nc.vector.tensor_scalar_mul(t[:, cb, 1:2], s2, 1.0 / N)
                for cb in bn_cbs:
                    for j in range(8):
                        nc.vector.bn_stats(
                            stats[:, cb, j, :], xbf[:, cb, ts(j, 512)]
                        )
                    nc.vector.bn_aggr(mv[:, cb, :], stats[:, cb, :, :])
                for cb in bn_cbs:
                    nc.vector.tensor_mul(
                        t[:, cb, 1:2], mv[:, cb, 0:1], mv[:, cb, 0:1]
                    )
                    nc.vector.tensor_add(
                        t[:, cb, 1:2], t[:, cb, 1:2], mv[:, cb, 1:2]
                    )
                    nc.vector.tensor_copy(t[:, cb, 0:1], mv[:, cb, 0:1])
                s["t"] = t

            def emit_a2(b):
                """Group aggregation -> A, B; diag tiles; scaled Wv; biases;
                scores rank-1 correction vectors."""
                s = st[b]
                t = s["t"]
                pg = psp.tile([GROUPS, 2], F32, tag="work", bufs=4, name=f"pg{b}")
                for cb in range(CB):
                    nc.tensor.matmul(
                        pg, gfwd[:, cb, :], t[:, cb, :],
                        start=(cb == 0), stop=(cb == CB - 1),
                    )
                gs = sbp.tile([GROUPS, 2], F32, tag="gs", bufs=2, name=f"gs{b}")
                pgs = sbp.tile([GROUPS, 2], F32, tag="pgs", bufs=2, name=f"pgs{b}")
                nc.vector.tensor_copy(pgs, pg)
                vtmp = sbp.tile([GROUPS, 1], F32, tag="vtmp", bufs=2, name=f"vt{b}")
                nc.vector.tensor_mul(vtmp, pgs[:, 0:1], pgs[:, 0:1])
                nc.vector.tensor_tensor(vtmp, pgs[:, 1:2], vtmp, op=OP.subtract)
                nc.vector.tensor_copy(gs[:, 0:1], pgs[:, 0:1])
                nc.scalar.activation(gs[:, 1:2], vtmp, AF.Sqrt, bias=eps_g)
                nc.vector.reciprocal(gs[:, 1:2], gs[:, 1:2])

                cst = sbp.tile([P, CB, 2], F32, tag="cst", bufs=2, name=f"cs{b}")
                for cb in range(CB):
                    pc = psp.tile([P, 2], F32, tag="work", bufs=4, name=f"pc{b}_{cb}")
                    nc.tensor.matmul(pc, gbwd[:, cb, :], gs, start=True, stop=True)
                    nc.vector.tensor_copy(cst[:, cb, :], pc)

                A_ = sbp.tile([P, CB], F32, tag="A_", bufs=2, name=f"A{b}")
                Bb = sbp.tile([P, CB], BF16, tag="Bb", bufs=2, name=f"B{b}")
                tmpB = sbp.tile([P, CB], F32, tag="tmpB", bufs=2, name=f"tB{b}")
                nc.vector.tensor_mul(A_, cst[:, :, 1], gam)
                nc.vector.tensor_mul(tmpB, cst[:, :, 0], A_)
                nc.vector.tensor_tensor(Bb, bet, tmpB, op=OP.subtract)

                # diag tiles D = diag(16*A) for the pixel-major hnT build
                Dt = sbp.tile([P, CB, P], BF16, tag="Dt", bufs=2, name=f"D{b}")
                s["Dt"] = Dt
                A16 = sbp.tile([P, CB], F32, tag="A16", bufs=2, name=f"A16{b}")
                nc.vector.tensor_scalar_mul(A16, A_, 16.0)
                for cb in range(CB):
                    nc.vector.tensor_scalar_mul(
                        Dt[:, cb, :], identbf, A16[:, cb : cb + 1]
                    )

                # Wv columns scaled by A
                wv_p = sbp.tile([P, CB, C], BF16, tag="wv_p", bufs=2, name=f"wv{b}")
                s["wv_p"] = wv_p
                for cb in range(CB):
                    nc.vector.tensor_scalar_mul(
                        wv_p[:, cb, :], wvt[:, cb, :], A_[:, cb : cb + 1]
                    )

                # v bias: bvb = bv + Wv@B, via DRAM round-trip to [P, CB]
                pb = psp.tile([1, C], F32, tag="work", bufs=4, name=f"pbv{b}")
                for cb in range(CB):
                    nc.tensor.matmul(
                        pb, Bb[:, cb : cb + 1], wvt[:, cb, :],
                        start=(cb == 0), stop=(cb == CB - 1),
                    )
                bvrow = sbp.tile([1, C], F32, tag="bvrow", bufs=2, name=f"bvr{b}")
                nc.vector.tensor_add(bvrow, pb, bvv)
                scr = drp.tile([C], F32, name=f"scrv{b}")
                nc.sync.dma_start(scr.rearrange("(a c) -> a c", a=1), bvrow)
                bvb = sbp.tile([P, CB], F32, tag="bvb", bufs=2, name=f"bvb{b}")
                nc.sync.dma_start(bvb, scr.rearrange("(cb p) -> p cb", p=P))
                s["bvb"] = bvb

                # scores rank-1 vectors (x256 scale):
                #   cq256 = 256*(Wq@B + bq), sq256 = 256*(Wq@rs), rs = A*N*mu
                rs16 = sbp.tile([P, CB], BF16, tag="rs16", bufs=2, name=f"rs{b}")
                rsf = sbp.tile([P, CB], F32, tag="rsf", bufs=2, name=f"rsf{b}")
                nc.vector.tensor_mul(rsf, A_, t[:, :, 0])
                nc.vector.tensor_scalar_mul(rs16, rsf, 16.0 * N)
                rows = {}
                for nm, wt, brow in (("q", wqt16, bq256r), ("k", wkt16, bk256r)):
                    pc1 = psp.tile([1, C], F32, tag="work", bufs=4,
                                   name=f"pc1{b}{nm}")
                    for cb in range(CB):
                        nc.tensor.matmul(
                            pc1, Bb[:, cb : cb + 1], wt[:, cb, :],
                            start=(cb == 0), stop=(cb == CB - 1),
                        )
                    crow = sbp.tile([1, C], BF16, tag=f"c{nm}row", bufs=2,
                                    name=f"c{nm}{b}")
                    tmpr = sbp.tile([1, C], F32, tag="tmpr", bufs=2,
                                    name=f"tr{b}{nm}")
                    nc.vector.tensor_scalar_mul(tmpr, pc1, 16.0)
                    nc.vector.tensor_add(crow, tmpr, brow)
                    rows[f"c{nm}"] = crow
                    ps1 = psp.tile([1, C], F32, tag="work", bufs=4,
                                   name=f"ps1{b}{nm}")
                    for cb in range(CB):
                        nc.tensor.matmul(
                            ps1, rs16[:, cb : cb + 1], wt[:, cb, :],
                            start=(cb == 0), stop=(cb == CB - 1),
                        )
                    srow = sbp.tile([1, C], BF16, tag=f"s{nm}row", bufs=2,
                                    name=f"s{nm}{b}")
                    nc.vector.tensor_copy(srow, ps1)
                    rows[f"s{nm}"] = srow
                rhs1 = sbp.tile([1, C], BF16, tag="rhs1", bufs=2, name=f"rh{b}")
                nc.vector.tensor_scalar_mul(rhs1, rows["ck"], float(N))
                nc.vector.tensor_add(rhs1, rhs1, rows["sk"])
                s["cq"], s["sq"], s["ck"] = rows["cq"], rows["sq"], rows["ck"]
                s["rhs1"] = rhs1

            def emit_gram(b):
                """hnT (pixel-major 16*A*x via PE diag matmul) -> Gram
                (upper triangle + mirrored blocks) -> T1 = (16Wq)^T G ->
                T1T (DMA transpose)."""
                s = st[b]
                xbf, Dt = s["xbf"], s["Dt"]
                hnT = sbp.tile([P, NTH, C], BF16, tag="hnT", bufs=1,
                               name=f"hnT{b}")
                pG = [
                    psp.tile([P, C - a * P], F32, tag="scores", bufs=4,
                             name=f"pG{b}_{a}")
                    for a in range(CB)
                ]
                for half in range(NT // NTH):
                    for ih in range(NTH):
                        i = half * NTH + ih
                        pT = psp.tile([P, C], F32, tag="work", bufs=4,
                                      name=f"pT{b}_{i}")
                        for cb in range(CB):
                            nc.tensor.matmul(
                                pT[:, ts(cb, P)], xbf[:, cb, ts(i, P)],
                                Dt[:, cb, :], start=True, stop=True,
                            )
                        nc.scalar.copy(hnT[:, ih, :], pT)
                    for ih in range(NTH):
                        i = half * NTH + ih
                        for a in range(CB):
                            nc.tensor.matmul(
                                pG[a], hnT[:, ih, ts(a, P)],
                                hnT[:, ih, a * P :],
                                start=(i == 0), stop=(i == NT - 1),
                            )
                Gb = sbp.tile([P, CB, C], BF16, tag="Gb", bufs=1, name=f"Gb{b}")
                for a in range(CB):
                    nc.scalar.copy(Gb[:, a, a * P :], pG[a])
                # mirror the 6 sub-diagonal blocks: G[b,a] = G[a,b]^T
                for a in range(CB):
                    for bb2 in range(a + 1, CB):
                        nc.sync.dma_start(
                            Gb[:, bb2, ts(a, P)],
                            Gb[:, a, ts(bb2, P)],
                            transpose=True,
                        )
                s["Gb"] = Gb

            def emit_t1t(b):
                """T1T[d, o] = sum_c G[d,c] 16Wq[o,c] — G is symmetric, so
                Gb blocks serve as lhsT directly; no transpose pass."""
                s = st[b]
                Gb = s["Gb"]
                T1T = sbp.tile([P, CB, C], BF16, tag="T1b", bufs=1, name=f"TT{b}")
                s["T1T"] = T1T
                for dcb in range(CB):
                    pT1 = psp.tile([P, C], F32, tag="work", bufs=4,
                                   name=f"pT1{b}_{dcb}")
                    for cb in range(CB):
                        nc.tensor.matmul(
                            pT1, Gb[:, cb, ts(dcb, P)], wqt16[:, cb, :],
                            start=(cb == 0), stop=(cb == CB - 1),
                        )
                    nc.scalar.copy(T1T[:, dcb, :], pT1)

            def emit_scores(b):
                """scores[o, e] = sum_d T1T[d, o] wkt16[d, e] + rank-1."""
                s = st[b]
                T1T = s["T1T"]
                cq, sq, ck, rhs1 = s["cq"], s["sq"], s["ck"], s["rhs1"]
                scores = [
                    psp.tile([P, C], F32, tag="scores", bufs=4, name=f"sc{b}_{cb}")
                    for cb in range(CB)
                ]
                s["scores"] = scores
                for ocb in range(CB):
                    for db in range(CB):
                        nc.tensor.matmul(
                            scores[ocb], T1T[:, db, ts(ocb, P)], wkt16[:, db, :],
                            start=(db == 0), stop=False,
                        )
                    nc.tensor.matmul(
                        scores[ocb], cq[:, ts(ocb, P)], rhs1,
                        start=False, stop=False,
                    )
                    nc.tensor.matmul(
                        scores[ocb], sq[:, ts(ocb, P)], ck,
                        start=False, stop=True,
                    )

            def emit_softmax(b):
                """Max-subtracted exp (x128), row sums -> rinv."""
                s = st[b]
                scores = s["scores"]
                e_sb = sbp.tile([P, CB, C], BF16, tag="e", bufs=1, name=f"e{b}")
                rinv = sbp.tile([P, CB], F32, tag="rinv", bufs=1, name=f"ri{b}")
                rmx = sbp.tile([P, CB], F32, tag="rmx", bufs=1, name=f"rm{b}")
                eb = sbp.tile([P, CB], F32, tag="eb", bufs=1, name=f"eb{b}")
                rsum = sbp.tile([P, CB], F32, tag="rsum", bufs=1, name=f"rs{b}")
                s["e"], s["rinv"] = e_sb, rinv
                for cb in range(CB):
                    nc.vector.reduce_max(
                        rmx[:, cb : cb + 1], scores[cb], axis=AX.X
                    )
                    nc.vector.tensor_scalar(
                        eb[:, cb : cb + 1], rmx[:, cb : cb + 1],
                        -SC2, LN128, op0=OP.mult, op1=OP.add,
                    )
                    nc.scalar.activation(
                        e_sb[:, cb, :], scores[cb], AF.Exp,
                        bias=eb[:, cb : cb + 1], scale=SC2,
                        accum_out=rsum[:, cb : cb + 1],
                    )
                    nc.vector.reciprocal(
                        rinv[:, cb : cb + 1], rsum[:, cb : cb + 1]
                    )

            def emit_t(b):
                """Transpose e -> eT via DMA transpose (bf16)."""
                s = st[b]
                e_sb = s["e"]
                eT = sbp.tile([P, CB, C], BF16, tag="eT", bufs=1, name=f"eT{b}")
                s["eT"] = eT
                for cb in range(CB):
                    for db in range(CB):
                        nc.sync.dma_start(
                            eT[:, db, ts(cb, P)],
                            e_sb[:, cb, ts(db, P)],
                            transpose=True,
                        )

            def emit_v(b, nsls):
                """v projection for the given pixel slices."""
                s = st[b]
                xbf, wv_p, bvb = s["xbf"], s["wv_p"], s["bvb"]
                if "vfull" not in s:
                    s["vfull"] = sbp.tile([P, CB, N], BF16, tag="vfull", bufs=1,
                                          name=f"v{b}")
                vfull = s["vfull"]
                for nsl in nsls:
                    for ob in range(CB):
                        pv = psp.tile([P, NSL], F32, tag="work", bufs=4,
                                      name=f"pv{b}{nsl}{ob}")
                        for cb in range(CB):
                            nc.tensor.matmul(
                                pv, wv_p[:, cb, ts(ob, P)],
                                xbf[:, cb, ts(nsl, NSL)],
                                start=(cb == 0), stop=(cb == CB - 1),
                            )
                        if (nsl * CB + ob) % 4 == 3:
                            nc.vector.tensor_scalar_add(
                                vfull[:, ob, ts(nsl, NSL)], pv,
                                bvb[:, ob : ob + 1],
                            )
                        else:
                            nc.scalar.add(
                                vfull[:, ob, ts(nsl, NSL)], pv,
                                bvb[:, ob : ob + 1],
                            )

            def emit_ef(b, early_free=False):
                s = st[b]
                eT, vfull, rinv = s["eT"], s["vfull"], s["rinv"]
                for nsl in range(NS):
                    pf_tag = "work" if (early_free and nsl >= NS - 1) else "scores"
                    xsl = sbp.tile([P, CB, NSL], BF16, tag="xsl", bufs=2,
                                   name=f"xs{b}_{nsl}")
                    for cb in range(CB):
                        nc.gpsimd.dma_start(
                            xsl[:, cb, :], xbview[b][:, cb, ts(nsl, NSL)]
                        )
                    ao = sbp.tile([P, CB, NSL], BF16, tag="ao", bufs=2,
                                  name=f"ao{b}_{nsl}")
                    for cb in range(CB):
                        pa = psp.tile([P, NSL], F32, tag="work", bufs=4,
                                      name=f"pa{b}{nsl}{cb}")
                        for db in range(CB):
                            nc.tensor.matmul(
                                pa, eT[:, db, ts(cb, P)],
                                vfull[:, db, ts(nsl, NSL)],
                                start=(db == 0), stop=(db == CB - 1),
                            )
                        nc.scalar.mul(ao[:, cb, :], pa, rinv[:, cb : cb + 1])

                    for ob in range(CB):
                        pf = psp.tile([P, NSL], F32, tag=pf_tag, bufs=4,
                                      name=f"pf{b}{nsl}{ob}")
                        for cb in range(CB):
                            nc.tensor.matmul(
                                pf, wot[:, cb, ts(ob, P)], ao[:, cb, :],
                                start=(cb == 0), stop=(cb == CB - 1),
                            )
                        yt = sbp.tile([P, NSL], F32, tag="yt", bufs=3,
                                      name=f"yt{b}{nsl}{ob}")
                        nc.vector.scalar_tensor_tensor(
                            yt, pf, bob[:, ob : ob + 1], xsl[:, ob, :],
                            op0=OP.add, op1=OP.add,
                        )
                        nc.sync.dma_start(yview[b][:, ob, ts(nsl, NSL)], yt)

            # ---- prologue ----
            emit_load(0)
            # HAM warm-up: keep TensorE busy/clocked through the prologue.
            zsb = sg.tile([P, NSL], BF16, name="zsb")
            nc.gpsimd.memset(zsb, 0.0)
            pdum = psp.tile([P, NSL], F32, tag="work", bufs=4, name="pdum")
            for i in range(24):
                nc.tensor.matmul(
                    pdum, zsb[:, :P], zsb, start=(i == 0), stop=False
                )
            for cb in range(CB):
                nc.tensor.matmul(
                    pdum, st[0]["xbf"][:, cb, ts(0, P)], zsb,
                    start=False, stop=(cb == CB - 1),
                )
            dsb = sg.tile([1, 1], F32, name="dsb")
            nc.vector.tensor_copy(dsb, pdum[0:1, 0:1])
            dscr = drp.tile([1], F32, name="dscr")
            nc.sync.dma_start(dscr.rearrange("(a c) -> a c", a=1), dsb)
            # ---- constants, loaded once ----
            gfwd = sg.tile([P, CB, GROUPS], F32)
            nc.sync.dma_start(gfwd, gfwd_d[:])
            gbwd = sg.tile([GROUPS, CB, P], F32)
            nc.sync.dma_start(gbwd, gbwd_d[:])
            wqt16 = sg.tile([P, CB, C], BF16)
            nc.sync.dma_start(wqt16, wqt16_d[:].rearrange("(cb p) o -> p cb o", p=P))
            wkt16 = sg.tile([P, CB, C], BF16)
            nc.sync.dma_start(wkt16, wkt16_d[:].rearrange("(cb p) o -> p cb o", p=P))
            wvt = sg.tile([P, CB, C], BF16)
            nc.sync.dma_start(wvt, wvt_d[:].rearrange("(cb p) o -> p cb o", p=P))
            wot = sg.tile([P, CB, C], BF16)
            nc.sync.dma_start(wot, wot_d[:].rearrange("(cb p) o -> p cb o", p=P))
            identbf = sg.tile([P, P], BF16)
            nc.sync.dma_start(identbf, identbf_d[:])
            gam = sg.tile([P, CB], F32)
            nc.sync.dma_start(gam, gamma_d[:].rearrange("(cb p) -> p cb", p=P))
            bet = sg.tile([P, CB], F32)
            nc.sync.dma_start(bet, beta_d[:].rearrange("(cb p) -> p cb", p=P))
            bob = sg.tile([P, CB], F32)
            nc.sync.dma_start(bob, bo_d[:].rearrange("(cb p) -> p cb", p=P))
            bq256r = sg.tile([1, C], F32)
            nc.sync.dma_start(bq256r, bq256_d[:].rearrange("(a c) -> a c", a=1))
            bk256r = sg.tile([1, C], F32)
            nc.sync.dma_start(bk256r, bk256_d[:].rearrange("(a c) -> a c", a=1))
            bvv = sg.tile([1, C], F32)
            nc.sync.dma_start(bvv, bv_d[:].rearrange("(a c) -> a c", a=1))
            eps_g = sg.tile([GROUPS, 1], F32)
            nc.vector.memset(eps_g, EPS)

            emit_stats(0, split=True)
            emit_a2(0)
            for b in range(BB):
                emit_gram(b)
                if b + 1 < BB:
                    emit_load(b + 1)
                emit_v(b, range(0, 3))
                emit_t1t(b)
                emit_v(b, range(3, 6))
                emit_scores(b)
                emit_softmax(b)
                emit_t(b)
                if b + 1 < BB:
                    emit_stats(b + 1)
                emit_v(b, range(6, NS))
                if b + 1 < BB:
                    emit_a2(b + 1)
                emit_ef(b, early_free=(b + 1 < BB))

    nc.finalize()
    return nc


def _get_nc():
    if "nc" not in _NC_CACHE:
        _NC_CACHE["nc"] = _build_nc()
    return _NC_CACHE["nc"]


def _make_consts():
    gfwd = np.zeros((P, CB, GROUPS), np.float32)
    gbwd = np.zeros((GROUPS, CB, P), np.float32)
    for cb in range(CB):
        for p in range(P):
            g = (cb * P + p) // 16
            gfwd[p, cb, g] = 1.0 / 16.0
            gbwd[g, cb, p] = 1.0
    return gfwd, gbwd


def kernel(x, gamma, beta, Wq, bq, Wk, bk, Wv, bv, Wo, bo):
    global LAST_RESULT
    from concourse.bass_utils import run_bass_kernel_spmd

    import ml_dtypes

    BF = ml_dtypes.bfloat16
    x = np.ascontiguousarray(np.asarray(x, np.float32)).reshape(16, C, N)
    xb16 = np.ascontiguousarray(x.astype(BF))
    gfwd, gbwd = _make_consts()
    shared = {
        "wqt16": np.ascontiguousarray(
            (np.asarray(Wq, np.float32).T * 16.0).astype(BF)
        ),
        "wkt16": np.ascontiguousarray(
            (np.asarray(Wk, np.float32).T * 16.0).astype(BF)
        ),
        "wvtb": np.ascontiguousarray(np.asarray(Wv, np.float32).T.astype(BF)),
        "wotb": np.ascontiguousarray(np.asarray(Wo, np.float32).T.astype(BF)),
        "bq256": np.ascontiguousarray(np.asarray(bq, np.float32) * 256.0),
        "bk256": np.ascontiguousarray(np.asarray(bk, np.float32) * 256.0),
        "bv": np.ascontiguousarray(np.asarray(bv, np.float32)),
        "bo": np.ascontiguousarray(np.asarray(bo, np.float32)),
        "gamma": np.ascontiguousarray(np.asarray(gamma, np.float32)),
        "beta": np.ascontiguousarray(np.asarray(beta, np.float32)),
        "gfwd": gfwd,
        "gbwd": gbwd,
        "identbf": np.ascontiguousarray(np.eye(P, dtype=np.float32).astype(BF)),
    }
    in_maps = [
        dict(shared, xsb=np.ascontiguousarray(xb16[BB * i : BB * (i + 1)]))
        for i in range(8)
    ]
    nc = _get_nc()
    import os

    trace = os.environ.get("KERNEL_TRACE") == "1"
    res = run_bass_kernel_spmd(nc, in_maps, core_ids=list(range(8)), trace=trace)
    LAST_RESULT = res
    y = np.concatenate([r["y"] for r in res.results], axis=0)
    return y.reshape(16, C, 64, 64)



# revision 5
# speedup vs baseline: 1.4708x; 1.4708x over previous
"""AttnBlock (channel attention over 64x64 maps) for Trainium2 — factored
epilogue edition.

Data-parallel over batch: 16 batches, 2 per core on 8 NeuronCores.
Per batch [C=512, N=4096], hn = A*x + B (GroupNorm folded to per-channel
affine):

  scores = q^T k with q = Wq hn + bq factorizes through the Gram matrix
    G = (A*x) @ (A*x)^T  (C x C):
      scores = (16Wq) G (16Wk)^T / 65536  + rank-1 corrections
    where the corrections come from the per-channel rowsums (free from the
    bn_stats means) and the GroupNorm shift B. This replaces the q-proj,
    k-proj and scores passes (3 full C*C*N GEMMs + their PSUM evictions)
    with: a diagonal matmul building pixel-major hnT (16k cyc), the Gram
    GEMM (upper triangle + mirrored blocks, ~41k cyc), and two C*C*C GEMMs
    (8k cyc each). All bf16 with fp32 PSUM accumulation.
  The epilogue factors the same way: out = Wo attn v with
    v = (Wv diag(A)) x + bvb 1^T collapses to
      out = M x + r 1^T,  M = Wo attn Wv diag(A),  r = Wo attn bvb + bo
    computed as R = (Wo attn)^T = e^T (rinv*Wo^T)  [8k cyc, e used as lhsT
    directly — no eT transpose pass], M^T = Wv R scaled by A at eviction
    [8k cyc], then ONE C*C*N application Y = M x + r + x [65k cyc] off the
    resident bf16 x. This replaces the v-proj / attn@v / out-proj trio
    (3 C*C*N GEMMs) and their DMA re-reads.
Batches are software-pipelined: batch1 stats/a2 run on vector during
batch0's Gram; batch1's Gram covers batch0's softmax; batch0's Y GEMM is
split around batch1's t1t/scores to cover the softmax/mirror seams.
"""

import sys

if "/opt/trn_rl_repo" not in sys.path:
    sys.path.insert(0, "/opt/trn_rl_repo")

import numpy as np

C = 512          # channels
N = 4096         # pixels (64*64)
BB = 2           # batches per core
P = 128          # partitions
CB = C // P      # 4 channel blocks
NT = N // P      # 32 pixel tiles of 128
NTH = 8          # pixel tiles per hnT chunk
NSL = 512        # pixel slice width (y phase)
NS = N // NSL    # 8 pixel slices
GROUPS = 32
EPS = 1e-6
SCALE = float(C) ** -0.5
SC2 = SCALE / 65536.0
LN128 = float(np.log(128.0))

_NC_CACHE = {}
LAST_RESULT = None


def _build_nc():
    import concourse.bacc as bacc
    import concourse.tile as tile
    from concourse import mybir
    from concourse.bass import ts

    F32 = mybir.dt.float32
    BF16 = mybir.dt.bfloat16
    AX = mybir.AxisListType
    AF = mybir.ActivationFunctionType
    OP = mybir.AluOpType

    nc = bacc.Bacc(None, target_bir_lowering=False, num_swdge_queues=4)

    xsb_d = nc.dram_tensor("xsb", [BB, C, N], BF16, kind="ExternalInput")
    wqt16_d = nc.dram_tensor("wqt16", [C, C], BF16, kind="ExternalInput")
    wkt16_d = nc.dram_tensor("wkt16", [C, C], BF16, kind="ExternalInput")
    wvt_d = nc.dram_tensor("wvtb", [C, C], BF16, kind="ExternalInput")
    wvr_d = nc.dram_tensor("wvrb", [C, C], BF16, kind="ExternalInput")
    wot_d = nc.dram_tensor("wotb", [C, C], BF16, kind="ExternalInput")
    bq256_d = nc.dram_tensor("bq256", [C], F32, kind="ExternalInput")
    bk256_d = nc.dram_tensor("bk256", [C], F32, kind="ExternalInput")
    bv_d = nc.dram_tensor("bv", [C], F32, kind="ExternalInput")
    bo_d = nc.dram_tensor("bo", [C], F32, kind="ExternalInput")
    gamma_d = nc.dram_tensor("gamma", [C], F32, kind="ExternalInput")
    beta_d = nc.dram_tensor("beta", [C], F32, kind="ExternalInput")
    gfwd_d = nc.dram_tensor("gfwd", [P, CB, GROUPS], F32, kind="ExternalInput")
    gbwd_d = nc.dram_tensor("gbwd", [GROUPS, CB, P], F32, kind="ExternalInput")
    identbf_d = nc.dram_tensor("identbf", [P, P], BF16, kind="ExternalInput")
    y_d = nc.dram_tensor("y", [BB, C, N], F32, kind="ExternalOutput")

    with tile.TileContext(nc) as tc:
        with (
            tc.tile_pool(name="singles", bufs=1) as sg,
            tc.tile_pool(name="sbp", bufs=1) as sbp,
            tc.tile_pool(name="psp", bufs=1, space="PSUM") as psp,
            tc.tile_pool(name="drp", bufs=1, space="DRAM") as drp,
        ):
            xbview = [xsb_d[b].rearrange("(cb p) n -> p cb n", p=P) for b in range(BB)]
            yview = [y_d[b].rearrange("(ob p) n -> p ob n", p=P) for b in range(BB)]
            st = [dict() for _ in range(BB)]  # per-batch tile state

            def emit_load(b):
                s = st[b]
                xbf = sbp.tile([P, CB, N], BF16, tag="xbf", bufs=2, name=f"xbf{b}")
                s["xbf"] = xbf
                for cb in range(CB):
                    nc.sync.dma_start(xbf[:, cb, :], xbview[b][:, cb, :])

            def emit_stats(b, split=False):
                """Per-channel [mean, E[x^2]] -> t."""
                s = st[b]
                xbf = s["xbf"]
                t = sbp.tile([P, CB, 2], F32, tag="t", bufs=2, name=f"t{b}")
                act_cbs = (0, 1) if split else ()
                bn_cbs = [cb for cb in range(CB) if cb not in act_cbs]
                stats = sbp.tile(
                    [P, CB, 8, 6], F32, tag="stats", bufs=2, name=f"st{b}"
                )
                mv = sbp.tile([P, CB, 2], F32, tag="mv", bufs=2, name=f"mv{b}")
                for cb in act_cbs:
                    sq = sbp.tile([P, N], F32, tag="scratch", bufs=1,
                                  name=f"sq{b}{cb}")
                    s1 = sbp.tile([P, 1], F32, tag="s1", bufs=2, name=f"s1{b}{cb}")
                    s2 = sbp.tile([P, 1], F32, tag="s2", bufs=2, name=f"s2{b}{cb}")
                    nc.scalar.activation(
                        sq, xbf[:, cb, :], AF.Copy, accum_out=s1
                    )
                    nc.scalar.activation(
                        sq, xbf[:, cb, :], AF.Square, accum_out=s2
                    )
                    nc.vector.tensor_scalar_mul(t[:, cb, 0:1], s1, 1.0 / N)
                    nc.vector.tensor_scalar_mul(t[:, cb, 1:2], s2, 1.0 / N)
                for cb in bn_cbs:
                    for j in range(8):
                        nc.vector.bn_stats(
                            stats[:, cb, j, :], xbf[:, cb, ts(j, 512)]
                        )
                    nc.vector.bn_aggr(mv[:, cb, :], stats[:, cb, :, :])
                for cb in bn_cbs:
                    nc.vector.tensor_mul(
                        t[:, cb, 1:2], mv[:, cb, 0:1], mv[:, cb, 0:1]
                    )
                    nc.vector.tensor_add(
                        t[:, cb, 1:2], t[:, cb, 1:2], mv[:, cb, 1:2]
                    )
                    nc.vector.tensor_copy(t[:, cb, 0:1], mv[:, cb, 0:1])
                s["t"] = t

            def emit_a2(b):
                """Group aggregation -> A, B; diag tiles; biases;
                scores rank-1 correction vectors."""
                s = st[b]
                t = s["t"]
                pg = psp.tile([GROUPS, 2], F32, tag="work", bufs=4, name=f"pg{b}")
                for cb in range(CB):
                    nc.tensor.matmul(
                        pg, gfwd[:, cb, :], t[:, cb, :],
                        start=(cb == 0), stop=(cb == CB - 1),
                    )
                gs = sbp.tile([GROUPS, 2], F32, tag="gs", bufs=2, name=f"gs{b}")
                pgs = sbp.tile([GROUPS, 2], F32, tag="pgs", bufs=2, name=f"pgs{b}")
                nc.vector.tensor_copy(pgs, pg)
                vtmp = sbp.tile([GROUPS, 1], F32, tag="vtmp", bufs=2, name=f"vt{b}")
                nc.vector.tensor_mul(vtmp, pgs[:, 0:1], pgs[:, 0:1])
                nc.vector.tensor_tensor(vtmp, pgs[:, 1:2], vtmp, op=OP.subtract)
                nc.vector.tensor_copy(gs[:, 0:1], pgs[:, 0:1])
                nc.scalar.activation(gs[:, 1:2], vtmp, AF.Sqrt, bias=eps_g)
                nc.vector.reciprocal(gs[:, 1:2], gs[:, 1:2])

                cst = sbp.tile([P, CB, 2], F32, tag="cst", bufs=2, name=f"cs{b}")
                for cb in range(CB):
                    pc = psp.tile([P, 2], F32, tag="work", bufs=4, name=f"pc{b}_{cb}")
                    nc.tensor.matmul(pc, gbwd[:, cb, :], gs, start=True, stop=True)
                    nc.vector.tensor_copy(cst[:, cb, :], pc)

                A_ = sbp.tile([P, CB], F32, tag="A_", bufs=2, name=f"A{b}")
                Bb = sbp.tile([P, CB], BF16, tag="Bb", bufs=2, name=f"B{b}")
                tmpB = sbp.tile([P, CB], F32, tag="tmpB", bufs=2, name=f"tB{b}")
                nc.vector.tensor_mul(A_, cst[:, :, 1], gam)
                nc.vector.tensor_mul(tmpB, cst[:, :, 0], A_)
                nc.vector.tensor_tensor(Bb, bet, tmpB, op=OP.subtract)
                s["A_"] = A_

                # diag tiles D = diag(16*A) for the pixel-major hnT build
                Dt = sbp.tile([P, CB, P], BF16, tag="Dt", bufs=2, name=f"D{b}")
                s["Dt"] = Dt
                A16 = sbp.tile([P, CB], F32, tag="A16", bufs=2, name=f"A16{b}")
                nc.vector.tensor_scalar_mul(A16, A_, 16.0)
                for cb in range(CB):
                    nc.vector.tensor_scalar_mul(
                        Dt[:, cb, :], identbf, A16[:, cb : cb + 1]
                    )

                # v bias: bvb = bv + Wv@B, via DRAM round-trip to [P, CB]
                pb = psp.tile([1, C], F32, tag="work", bufs=4, name=f"pbv{b}")
                for cb in range(CB):
                    nc.tensor.matmul(
                        pb, Bb[:, cb : cb + 1], wvt[:, cb, :],
                        start=(cb == 0), stop=(cb == CB - 1),
                    )
                bvrow = sbp.tile([1, C], F32, tag="bvrow", bufs=2, name=f"bvr{b}")
                nc.vector.tensor_add(bvrow, pb, bvv)
                scr = drp.tile([C], F32, name=f"scrv{b}")
                nc.sync.dma_start(scr.rearrange("(a c) -> a c", a=1), bvrow)
                bvb = sbp.tile([P, CB], F32, tag="bvb", bufs=2, name=f"bvb{b}")
                nc.sync.dma_start(bvb, scr.rearrange("(cb p) -> p cb", p=P))
                bvb16 = sbp.tile([P, CB], BF16, tag="bvb16", bufs=2,
                                 name=f"bvb16{b}")
                nc.vector.tensor_copy(bvb16, bvb)
                s["bvb"] = bvb16

                # scores rank-1 vectors (x256 scale):
                #   cq256 = 256*(Wq@B + bq), sq256 = 256*(Wq@rs), rs = A*N*mu
                rs16 = sbp.tile([P, CB], BF16, tag="rs16", bufs=2, name=f"rs{b}")
                rsf = sbp.tile([P, CB], F32, tag="rsf", bufs=2, name=f"rsf{b}")
                nc.vector.tensor_mul(rsf, A_, t[:, :, 0])
                nc.vector.tensor_scalar_mul(rs16, rsf, 16.0 * N)
                rows = {}
                for nm, wt, brow in (("q", wqt16, bq256r), ("k", wkt16, bk256r)):
                    pc1 = psp.tile([1, C], F32, tag="work", bufs=4,
                                   name=f"pc1{b}{nm}")
                    for cb in range(CB):
                        nc.tensor.matmul(
                            pc1, Bb[:, cb : cb + 1], wt[:, cb, :],
                            start=(cb == 0), stop=(cb == CB - 1),
                        )
                    crow = sbp.tile([1, C], BF16, tag=f"c{nm}row", bufs=2,
                                    name=f"c{nm}{b}")
                    tmpr = sbp.tile([1, C], F32, tag="tmpr", bufs=2,
                                    name=f"tr{b}{nm}")
                    nc.vector.tensor_scalar_mul(tmpr, pc1, 16.0)
                    nc.vector.tensor_add(crow, tmpr, brow)
                    rows[f"c{nm}"] = crow
                    ps1 = psp.tile([1, C], F32, tag="work", bufs=4,
                                   name=f"ps1{b}{nm}")
                    for cb in range(CB):
                        nc.tensor.matmul(
                            ps1, rs16[:, cb : cb + 1], wt[:, cb, :],
                            start=(cb == 0), stop=(cb == CB - 1),
                        )
                    srow = sbp.tile([1, C], BF16, tag=f"s{nm}row", bufs=2,
                                    name=f"s{nm}{b}")
                    nc.vector.tensor_copy(srow, ps1)
                    rows[f"s{nm}"] = srow
                rhs1 = sbp.tile([1, C], BF16, tag="rhs1", bufs=2, name=f"rh{b}")
                nc.vector.tensor_scalar_mul(rhs1, rows["ck"], float(N))
                nc.vector.tensor_add(rhs1, rhs1, rows["sk"])
                s["cq"], s["sq"], s["ck"] = rows["cq"], rows["sq"], rows["ck"]
                s["rhs1"] = rhs1

            def emit_gram(b):
                """hnT (pixel-major 16*A*x via PE diag matmul) -> Gram
                (upper triangle + mirrored blocks)."""
                s = st[b]
                xbf, Dt = s["xbf"], s["Dt"]
                hnT = sbp.tile([P, NTH, C], BF16, tag="hnT", bufs=1,
                               name=f"hnT{b}")
                pG = [
                    psp.tile([P, C - a * P], F32, tag="scores", bufs=4,
                             name=f"pG{b}_{a}")
                    for a in range(CB)
                ]
                for half in range(NT // NTH):
                    for ih in range(NTH):
                        i = half * NTH + ih
                        pT = psp.tile([P, C], F32, tag="work", bufs=4,
                                      name=f"pT{b}_{i}")
                        for cb in range(CB):
                            nc.tensor.matmul(
                                pT[:, ts(cb, P)], xbf[:, cb, ts(i, P)],
                                Dt[:, cb, :], start=True, stop=True,
                            )
                        nc.scalar.copy(hnT[:, ih, :], pT)
                    for ih in range(NTH):
                        i = half * NTH + ih
                        for a in range(CB):
                            nc.tensor.matmul(
                                pG[a], hnT[:, ih, ts(a, P)],
                                hnT[:, ih, a * P :],
                                start=(i == 0), stop=(i == NT - 1),
                            )
                Gb = sbp.tile([P, CB, C], BF16, tag="Gb", bufs=1, name=f"Gb{b}")
                for a in range(CB):
                    nc.scalar.copy(Gb[:, a, a * P :], pG[a])
                # mirror the 6 sub-diagonal blocks: G[b,a] = G[a,b]^T
                for a in range(CB):
                    for bb2 in range(a + 1, CB):
                        nc.sync.dma_start(
                            Gb[:, bb2, ts(a, P)],
                            Gb[:, a, ts(bb2, P)],
                            transpose=True,
                        )
                s["Gb"] = Gb

            def emit_t1t(b):
                """T1T[d, o] = sum_c G[d,c] 16Wq[o,c] — G is symmetric, so
                Gb blocks serve as lhsT directly; no transpose pass."""
                s = st[b]
                Gb = s["Gb"]
                T1T = sbp.tile([P, CB, C], BF16, tag="T1b", bufs=1, name=f"TT{b}")
                s["T1T"] = T1T
                for dcb in range(CB):
                    pT1 = psp.tile([P, C], F32, tag="work", bufs=4,
                                   name=f"pT1{b}_{dcb}")
                    for cb in range(CB):
                        nc.tensor.matmul(
                            pT1, Gb[:, cb, ts(dcb, P)], wqt16[:, cb, :],
                            start=(cb == 0), stop=(cb == CB - 1),
                        )
                    nc.scalar.copy(T1T[:, dcb, :], pT1)

            def emit_scores(b):
                """scores[o, e] = sum_d T1T[d, o] wkt16[d, e] + rank-1."""
                s = st[b]
                T1T = s["T1T"]
                cq, sq, ck, rhs1 = s["cq"], s["sq"], s["ck"], s["rhs1"]
                scores = [
                    psp.tile([P, C], F32, tag="scores", bufs=4, name=f"sc{b}_{cb}")
                    for cb in range(CB)
                ]
                s["scores"] = scores
                for ocb in range(CB):
                    for db in range(CB):
                        nc.tensor.matmul(
                            scores[ocb], T1T[:, db, ts(ocb, P)], wkt16[:, db, :],
                            start=(db == 0), stop=False,
                        )
                    nc.tensor.matmul(
                        scores[ocb], cq[:, ts(ocb, P)], rhs1,
                        start=False, stop=False,
                    )
                    nc.tensor.matmul(
                        scores[ocb], sq[:, ts(ocb, P)], ck,
                        start=False, stop=True,
                    )

            def emit_softmax(b):
                """Max-subtracted exp (x128), row sums -> rinv."""
                s = st[b]
                scores = s["scores"]
                e_sb = sbp.tile([P, CB, C], BF16, tag="e", bufs=1, name=f"e{b}")
                rinv = sbp.tile([P, CB], F32, tag="rinv", bufs=1, name=f"ri{b}")
                rmx = sbp.tile([P, CB], F32, tag="rmx", bufs=1, name=f"rm{b}")
                eb = sbp.tile([P, CB], F32, tag="eb", bufs=1, name=f"eb{b}")
                rsum = sbp.tile([P, CB], F32, tag="rsum", bufs=1, name=f"rs{b}")
                s["e"], s["rinv"] = e_sb, rinv
                for cb in range(CB):
                    nc.vector.reduce_max(
                        rmx[:, cb : cb + 1], scores[cb], axis=AX.X
                    )
                    nc.vector.tensor_scalar(
                        eb[:, cb : cb + 1], rmx[:, cb : cb + 1],
                        -SC2, LN128, op0=OP.mult, op1=OP.add,
                    )
                    nc.scalar.activation(
                        e_sb[:, cb, :], scores[cb], AF.Exp,
                        bias=eb[:, cb : cb + 1], scale=SC2,
                        accum_out=rsum[:, cb : cb + 1],
                    )
                    nc.vector.reciprocal(
                        rinv[:, cb : cb + 1], rsum[:, cb : cb + 1]
                    )

            def emit_m(b):
                """R = (Wo attn)^T = e^T (rinv*Wo^T); r = R^T bvb + bo;
                M^T[e,o] = A[e] * sum_d Wv[d,e] R[d,o]."""
                s = st[b]
                e_sb, rinv, bvb, A_ = s["e"], s["rinv"], s["bvb"], s["A_"]
                wotr = sbp.tile([P, CB, C], BF16, tag="wotr", bufs=2,
                                name=f"wr{b}")
                for cb in range(CB):
                    nc.vector.tensor_scalar_mul(
                        wotr[:, cb, :], wot[:, cb, :], rinv[:, cb : cb + 1]
                    )
                Rb = sbp.tile([P, CB, C], BF16, tag="Rb", bufs=2, name=f"Rb{b}")
                for db in range(CB):
                    pR = psp.tile([P, C], F32, tag="work", bufs=4,
                                  name=f"pR{b}{db}")
                    for cb in range(CB):
                        nc.tensor.matmul(
                            pR, e_sb[:, cb, ts(db, P)], wotr[:, cb, :],
                            start=(cb == 0), stop=(cb == CB - 1),
                        )
                    nc.scalar.copy(Rb[:, db, :], pR)
                # r[o] = sum_d R[d, o] bvb[d] + bo, per-partition layout
                pr = psp.tile([P, CB], F32, tag="work", bufs=4, name=f"pr{b}")
                for ob in range(CB):
                    for db in range(CB):
                        nc.tensor.matmul(
                            pr[:, ob : ob + 1], Rb[:, db, ts(ob, P)],
                            bvb[:, db : db + 1],
                            start=(db == 0), stop=(db == CB - 1),
                        )
                rb = sbp.tile([P, CB], F32, tag="rb", bufs=2, name=f"rv{b}")
                nc.vector.tensor_add(rb, pr, bob)
                s["rb"] = rb
                MtT = sbp.tile([P, CB, C], BF16, tag="MtT", bufs=2,
                               name=f"Mt{b}")
                for eb2 in range(CB):
                    pM = psp.tile([P, C], F32, tag="work", bufs=4,
                                  name=f"pM{b}{eb2}")
                    for db in range(CB):
                        nc.tensor.matmul(
                            pM, wvr[:, db, ts(eb2, P)], Rb[:, db, :],
                            start=(db == 0), stop=(db == CB - 1),
                        )
                    nc.scalar.mul(MtT[:, eb2, :], pM, A_[:, eb2 : eb2 + 1])
                s["MtT"] = MtT

            def emit_y(b, nsls):
                """Y = M x + r 1^T + x for the given pixel slices."""
                s = st[b]
                xbf, MtT, rb = s["xbf"], s["MtT"], s["rb"]
                for nsl in nsls:
                    for ob in range(CB):
                        pf = psp.tile([P, NSL], F32, tag="work", bufs=4,
                                      name=f"pf{b}{nsl}{ob}")
                        for eb2 in range(CB):
                            nc.tensor.matmul(
                                pf, MtT[:, eb2, ts(ob, P)],
                                xbf[:, eb2, ts(nsl, NSL)],
                                start=(eb2 == 0), stop=(eb2 == CB - 1),
                            )
                        yt = sbp.tile([P, NSL], F32, tag="yt", bufs=3,
                                      name=f"yt{b}{nsl}{ob}")
                        nc.vector.scalar_tensor_tensor(
                            yt, pf, rb[:, ob : ob + 1],
                            xbf[:, ob, ts(nsl, NSL)],
                            op0=OP.add, op1=OP.add,
                        )
                        nc.sync.dma_start(yview[b][:, ob, ts(nsl, NSL)], yt)

            # ---- prologue ----
            emit_load(0)
            # HAM warm-up: keep TensorE busy/clocked through the prologue.
            zsb = sg.tile([P, NSL], BF16, name="zsb")
            nc.gpsimd.memset(zsb, 0.0)
            pdum = psp.tile([P, NSL], F32, tag="work", bufs=4, name="pdum")
            for i in range(24):
                nc.tensor.matmul(
                    pdum, zsb[:, :P], zsb, start=(i == 0), stop=False
                )
            for cb in range(CB):
                nc.tensor.matmul(
                    pdum, st[0]["xbf"][:, cb, ts(0, P)], zsb,
                    start=False, stop=(cb == CB - 1),
                )
            dsb = sg.tile([1, 1], F32, name="dsb")
            nc.vector.tensor_copy(dsb, pdum[0:1, 0:1])
            dscr = drp.tile([1], F32, name="dscr")
            nc.sync.dma_start(dscr.rearrange("(a c) -> a c", a=1), dsb)
            # ---- constants, loaded once ----
            gfwd = sg.tile([P, CB, GROUPS], F32)
            nc.sync.dma_start(gfwd, gfwd_d[:])
            gbwd = sg.tile([GROUPS, CB, P], F32)
            nc.sync.dma_start(gbwd, gbwd_d[:])
            wqt16 = sg.tile([P, CB, C], BF16)
            nc.sync.dma_start(wqt16, wqt16_d[:].rearrange("(cb p) o -> p cb o", p=P))
            wkt16 = sg.tile([P, CB, C], BF16)
            nc.sync.dma_start(wkt16, wkt16_d[:].rearrange("(cb p) o -> p cb o", p=P))
            wvt = sg.tile([P, CB, C], BF16)
            nc.sync.dma_start(wvt, wvt_d[:].rearrange("(cb p) o -> p cb o", p=P))
            wvr = sg.tile([P, CB, C], BF16)
            nc.sync.dma_start(wvr, wvr_d[:].rearrange("(cb p) o -> p cb o", p=P))
            wot = sg.tile([P, CB, C], BF16)
            nc.sync.dma_start(wot, wot_d[:].rearrange("(cb p) o -> p cb o", p=P))
            identbf = sg.tile([P, P], BF16)
            nc.sync.dma_start(identbf, identbf_d[:])
            gam = sg.tile([P, CB], F32)
            nc.sync.dma_start(gam, gamma_d[:].rearrange("(cb p) -> p cb", p=P))
            bet = sg.tile([P, CB], F32)
            nc.sync.dma_start(bet, beta_d[:].rearrange("(cb p) -> p cb", p=P))
            bob = sg.tile([P, CB], F32)
            nc.sync.dma_start(bob, bo_d[:].rearrange("(cb p) -> p cb", p=P))
            bq256r = sg.tile([1, C], F32)
            nc.sync.dma_start(bq256r, bq256_d[:].rearrange("(a c) -> a c", a=1))
            bk256r = sg.tile([1, C], F32)
            nc.sync.dma_start(bk256r, bk256_d[:].rearrange("(a c) -> a c", a=1))
            bvv = sg.tile([1, C], F32)
            nc.sync.dma_start(bvv, bv_d[:].rearrange("(a c) -> a c", a=1))
            eps_g = sg.tile([GROUPS, 1], F32)
            nc.vector.memset(eps_g, EPS)

            # ---- pipelined schedule (BB=2) ----
            emit_stats(0, split=True)
            emit_a2(0)
            emit_load(1)
            emit_stats(1)          # vector stats(1) run under gram(0) PE work
            emit_gram(0)
            emit_a2(1)             # small PE bits; waits on stats(1)
            emit_t1t(0)
            emit_scores(0)
            emit_softmax(0)
            emit_gram(1)           # PE-heavy; covers softmax(0) latency
            emit_m(0)
            emit_y(0, range(0, 4))
            emit_t1t(1)            # mirror-DMA seam covered by m(0)/y(0)
            emit_scores(1)
            emit_y(0, range(4, NS))  # covers softmax(1) latency
            emit_softmax(1)
            emit_m(1)
            emit_y(1, range(NS))

    nc.finalize()
    return nc


def _get_nc():
    if "nc" not in _NC_CACHE:
        _NC_CACHE["nc"] = _build_nc()
    return _NC_CACHE["nc"]


def _make_consts():
    gfwd = np.zeros((P, CB, GROUPS), np.float32)
    gbwd = np.zeros((GROUPS, CB, P), np.float32)
    for cb in range(CB):
        for p in range(P):
            g = (cb * P + p) // 16
            gfwd[p, cb, g] = 1.0 / 16.0
            gbwd[g, cb, p] = 1.0
    return gfwd, gbwd


def kernel(x, gamma, beta, Wq, bq, Wk, bk, Wv, bv, Wo, bo):
    global LAST_RESULT
    from concourse.bass_utils import run_bass_kernel_spmd

    import ml_dtypes

    BF = ml_dtypes.bfloat16
    x = np.ascontiguousarray(np.asarray(x, np.float32)).reshape(16, C, N)
    xb16 = np.ascontiguousarray(x.astype(BF))
    gfwd, gbwd = _make_consts()
    shared = {
        "wqt16": np.ascontiguousarray(
            (np.asarray(Wq, np.float32).T * 16.0).astype(BF)
        ),
        "wkt16": np.ascontiguousarray(
            (np.asarray(Wk, np.float32).T * 16.0).astype(BF)
        ),
        "wvtb": np.ascontiguousarray(np.asarray(Wv, np.float32).T.astype(BF)),
        "wvrb": np.ascontiguousarray(np.asarray(Wv, np.float32).astype(BF)),
        "wotb": np.ascontiguousarray(np.asarray(Wo, np.float32).T.astype(BF)),
        "bq256": np.ascontiguousarray(np.asarray(bq, np.float32) * 256.0),
        "bk256": np.ascontiguousarray(np.asarray(bk, np.float32) * 256.0),
        "bv": np.ascontiguousarray(np.asarray(bv, np.float32)),
        "bo": np.ascontiguousarray(np.asarray(bo, np.float32)),
        "gamma": np.ascontiguousarray(np.asarray(gamma, np.float32)),
        "beta": np.ascontiguousarray(np.asarray(beta, np.float32)),
        "gfwd": gfwd,
        "gbwd": gbwd,
        "identbf": np.ascontiguousarray(np.eye(P, dtype=np.float32).astype(BF)),
    }
    in_maps = [
        dict(shared, xsb=np.ascontiguousarray(xb16[BB * i : BB * (i + 1)]))
        for i in range(8)
    ]
    nc = _get_nc()
    import os

    trace = os.environ.get("KERNEL_TRACE") == "1"
    res = run_bass_kernel_spmd(nc, in_maps, core_ids=list(range(8)), trace=trace)
    LAST_RESULT = res
    y = np.concatenate([r["y"] for r in res.results], axis=0)
    return y.reshape(16, C, 64, 64)
